# revision 29
# baseline (speedup 1.0000x reference)
"""Distributed causal multi-head attention for 8 TRN2 NeuronCores.

Problem: x[4,2048,1024], per-head Q/K/V [16,64,1024], O [1024,1024].
  q,k,v = per-head projections of x; scores = q@k^T (no 1/sqrt(d));
  causal softmax; z = attn@v; out = z @ O^T.

Sharding (head-parallel): core j owns heads {2j, 2j+1} for ALL batches.

Structure (vs the 2-A2A baseline):
  - Single-head attention units (b, mq, h) so the z-exchange splits into
    FOUR AllToAlls (512 KB each: 1a/1b = macros {0,2} heads A/B, 2a/2b =
    macros {1,3}): 1a/1b hide under phase II head-A attention, 2a hides
    under phase II head-B, 2b hides under the O-projection of rows 0-511.
  - Causal diagonal trimmed at 128-col granularity: scores/PV matmuls and
    exp only cover q >= 128*d within the diagonal 512-block.
  - Causal mask applied IN PSUM via one extra 128-col matmul per diagonal
    k-tile: tri[p,i]=[p<=i] (stationary) x mneg[p,j]=-1000*[p>j] (moving)
    accumulates -1000*max(0,i-j) onto the boundary subtile, so exp
    underflows to exactly 0 above the diagonal. No gpsimd affine_select
    on the critical path.
  - PV ones-column trick with l at psum partition 0 (v tile layout
    [1|vA|1|vB]): reciprocal runs directly on pz[0:1,:] (DVE reads psum,
    offsets match), no l-row DMA, no zcp copy.
  - PV of group g-1 is emitted after scores+exp of group g (software
    pipelining) so the PE never waits on the exp of the group it just
    computed.
"""

import os

import numpy as np
import ml_dtypes

import concourse.mybir as mybir
import concourse.tile as tile
from concourse import bacc
from concourse.bass_utils import run_bass_kernel_spmd

BF16 = mybir.dt.bfloat16
F32 = mybir.dt.float32
FP16 = mybir.dt.float16

B, M, NH, DH = 4, 1024, 16, 64
NCORES = 8
S = 2048
NM = S // 512          # 512-wide q-macros per batch
CH = (B * S) // NCORES  # output rows per core (1024)
CH2 = CH // 2

Exp = mybir.ActivationFunctionType.Exp
Copy = mybir.ActivationFunctionType.Copy


def build():
    nc = bacc.Bacc("TRN2", target_bir_lowering=False, debug=False, num_devices=NCORES)
    xt_ext = nc.dram_tensor("xt", [B, M, S], FP16, kind="ExternalInput")
    wqk_ext = nc.dram_tensor("wqk", [M, 256], FP16, kind="ExternalInput")
    wv_ext = nc.dram_tensor("wv", [M, 128], FP16, kind="ExternalInput")
    ot_ext = nc.dram_tensor("ot", [M, M], BF16, kind="ExternalInput")
    msk_ext = nc.dram_tensor("msk", [128, 256], FP16, kind="ExternalInput")
    out_ext = nc.dram_tensor("out", [CH, M], F32, kind="ExternalOutput")

    with (
        tile.TileContext(nc) as tc,
        tc.tile_pool(name="wpool", bufs=1) as wpool,
        tc.tile_pool(name="xt", bufs=18) as xt_pool,
        tc.tile_pool(name="qk", bufs=1) as qk_pool,
        tc.tile_pool(name="kz", bufs=1) as kz_pool,
        tc.tile_pool(name="vp", bufs=1) as v_pool,
        tc.tile_pool(name="ep", bufs=6) as e_pool,
        tc.tile_pool(name="zp", bufs=6) as z_pool,
        tc.tile_pool(name="zr", bufs=1) as zr_pool,
        tc.tile_pool(name="ob", bufs=2) as ob_pool,
        tc.tile_pool(name="nrm", bufs=3) as nrm_pool,
        tc.tile_pool(name="ps_sc", bufs=2, space="PSUM") as ps_sc,
        tc.tile_pool(name="ps_z", bufs=2, space="PSUM") as ps_z,
        tc.tile_pool(name="ps_gen", bufs=2, space="PSUM") as ps_gen,
        tc.tile_pool(name="dram", bufs=1, space="DRAM") as dram,
    ):
        # ---- weights (resident) ----
        wqk_sb, wv_sb, ot_sb = [], [], []
        for m in range(8):
            t = wpool.tile([128, 256], FP16, name=f"wqk{m}", tag=f"wqk{m}")
            nc.sync.dma_start(t[:], wqk_ext[128 * m:128 * (m + 1), :])
            wqk_sb.append(t)
            t = wpool.tile([128, 128], FP16, name=f"wv{m}", tag=f"wv{m}")
            nc.gpsimd.dma_start(t[:], wv_ext[128 * m:128 * (m + 1), :])
            wv_sb.append(t)
            t = wpool.tile([128, 1024], BF16, name=f"ot{m}", tag=f"ot{m}")
            ot_sb.append(t)

        # ---- causal-mask constant tiles (host-prepared) ----
        # tri[p,i] = 1 if p <= i ; mneg[p,j] = -1000 if p > j else 0.
        # matmul(tri, mneg) accumulates -1000*max(0, i-j) onto scores psum.
        msk = wpool.tile([128, 256], FP16, name="msk", tag="msk")
        nc.gpsimd.dma_start(msk[:], msk_ext[:, :])
        tri = msk[:, 0:128]
        mneg = msk[:, 128:256]

        # ---- exchange buffers: [rank, 64 v-dims, 512 q] per head/phase ----
        def dbuf(name):
            return dram.tile([NCORES, 64, 512], BF16, name=name)

        a2a = {k: (dbuf(f"in_{k}"), dbuf(f"out_{k}"))
               for k in ("1a", "1b", "2a", "2b")}

        qk_sb = {}   # (b, mq) -> [128, 512] fp16 qT, rows = (qA dims | qB dims)
        kz_sb = {}   # (h, b, mk) -> [128, 512] fp16 zero-padded per-head kT
        v_sb = {}    # (b, kt) -> [128, 130] bf16: [1 | vA(64) | 1 | vB(64)]

        def emit_xt(b, mq, xt_eng=None):
            xts = []
            for m in range(8):
                t = xt_pool.tile([128, 512], FP16, name="xtc")
                eng = xt_eng if xt_eng is not None else nc.sync
                eng.dma_start(
                    t[:], xt_ext[b, 128 * m:128 * (m + 1), 512 * mq:512 * (mq + 1)]
                )
                xts.append(t)
            return xts

        def _emit_v_tile(b, mq, stl, ps):
            kt = 4 * mq + stl
            # [1 | vA(64) | 1 | vB(64)]: ones column -> PV psum row 0 = l,
            # z dims land on psum rows 1..64
            vt = v_pool.tile([128, 130], BF16, name=f"v_{b}_{kt}", tag=f"v_{b}_{kt}")
            nc.gpsimd.memset(vt[:, 0:1], 1.0)
            nc.gpsimd.memset(vt[:, 65:66], 1.0)
            nc.vector.tensor_copy(
                vt[:].rearrange("p (g c) -> p g c", g=2)[:, :, 1:65],
                ps[:].rearrange("p (g c) -> p g c", g=2),
            )
            v_sb[(b, kt)] = vt

        def emit_proj(b, mq, xts):
            # interleave the 512-wide qk matmuls with the 128-wide v matmuls
            # so each one's LDWEIGHTS hides under the other's execution
            for ct in range(2):
                ps = ps_gen.tile([128, 512], F32, name="psqk", tag="gen")
                psv = ps_gen.tile([128, 128], F32, name="psv", tag="gen")
                for m in range(8):
                    nc.tensor.matmul(
                        ps[:],
                        wqk_sb[m][:, 128 * ct:128 * (ct + 1)],
                        xts[m][:],
                        start=(m == 0),
                        stop=(m == 7),
                    )
                    nc.tensor.matmul(
                        psv[:],
                        xts[m][:, 128 * ct:128 * (ct + 1)],
                        wv_sb[m][:],
                        start=(m == 0),
                        stop=(m == 7),
                    )
                _emit_v_tile(b, mq, ct, psv)
                if ct == 0:
                    t = qk_pool.tile(
                        [128, 512], FP16, name=f"qk_{b}_{mq}", tag=f"qk_{b}_{mq}"
                    )
                    nc.vector.tensor_copy(t[:], ps[:])
                    qk_sb[(b, mq)] = t
                else:
                    for h in range(2):
                        kz = kz_pool.tile(
                            [128, 512], FP16, name=f"kz{h}_{b}_{mq}",
                            tag=f"kz{h}_{b}_{mq}",
                        )
                        nc.vector.memset(kz[64 - 64 * h:128 - 64 * h, :], 0.0)
                        nc.scalar.activation(
                            kz[64 * h:64 * (h + 1), :],
                            ps[64 * h:64 * (h + 1), :],
                            Copy,
                        )
                        kz_sb[(h, b, mq)] = kz
            for stl in (2, 3):
                ps = ps_gen.tile([128, 128], F32, name="psv", tag="gen")
                for m in range(8):
                    nc.tensor.matmul(
                        ps[:],
                        xts[m][:, 128 * stl:128 * (stl + 1)],
                        wv_sb[m][:],
                        start=(m == 0),
                        stop=(m == 7),
                    )
                _emit_v_tile(b, mq, stl, ps)

        def emit_attn(b, mq, h, pz_pool=None):
            """Single-head attention unit -> zt DMA into its exchange slice.

            pz_pool: the macro-3 phase (no projections running) borrows the
            idle ps_gen banks for half its pz tiles, giving a 4-deep pz
            rotation. A collective_compute occupies the gpsimd queue until
            the collective completes, stalling partition_broadcasts of the
            following units; the deeper rotation keeps the PE fed meanwhile.
            """
            nk = 4 * (mq + 1)
            buf = a2a[("1" if mq % 2 == 0 else "2") + ("a" if h == 0 else "b")][0]
            rank = 2 * b + mq // 2
            if pz_pool is None:
                pz_pool = ps_z
            pz = pz_pool.tile(
                [128, 512], F32, name="pz",
                tag="pz" if pz_pool is ps_z else "gen",
            )
            qk = qk_sb[(b, mq)]
            pending = None

            def emit_pv(p):
                e, spans = p
                for kt, kk, q0 in spans:
                    vt = v_sb[(b, kt)]
                    nc.tensor.matmul(
                        pz[0:65, q0:512],
                        vt[:, 65 * h:65 * h + 65],
                        e[:, 512 * kk + q0:512 * (kk + 1)],
                        start=(kt == 0),
                        stop=(kt == nk - 1),
                        skip_group_check=True,
                    )

            for g in range(nk // 2):
                psc = ps_sc.tile([128, 1024], F32, name="psc", tag="sc")
                spans = []
                for kk in range(2):
                    kt = 2 * g + kk
                    mk, ktl = kt // 4, kt % 4
                    d = kt - 4 * mq
                    q0 = 128 * d if d >= 0 else 0
                    nc.tensor.matmul(
                        psc[:, 512 * kk + q0:512 * (kk + 1)],
                        kz_sb[(h, b, mk)][:, 128 * ktl:128 * (ktl + 1)],
                        qk[:, q0:512],
                        start=True,
                        stop=(d < 0),
                        skip_group_check=True,
                    )
                    if d >= 0:
                        # subtract 1000*(k_local - q_local) on the boundary
                        # 128x128 subtile; exp underflows to exact 0 there
                        nc.tensor.matmul(
                            psc[:, 512 * kk + q0:512 * kk + q0 + 128],
                            tri,
                            mneg,
                            start=False,
                            stop=True,
                            skip_group_check=True,
                        )
                    spans.append((kt, kk, q0))
                e = e_pool.tile([128, 1024], BF16, name="etile")
                # one exp per group when the mid-tile garbage span is small
                # (cheaper than a second instruction's fixed cost)
                if spans[1][2] <= 128:
                    q0a = spans[0][2]
                    nc.scalar.activation(e[:, q0a:1024], psc[:, q0a:1024], Exp)
                else:
                    for kt, kk, q0 in spans:
                        nc.scalar.activation(
                            e[:, 512 * kk + q0:512 * (kk + 1)],
                            psc[:, 512 * kk + q0:512 * (kk + 1)],
                            Exp,
                        )
                if pending is not None:
                    emit_pv(pending)
                pending = (e, spans)
            emit_pv(pending)

            # normalize: l = pz row 0 (psum partition 0); rec/bc/mul with
            # matching partition offsets throughout
            rec = nrm_pool.tile([1, 512], F32, name="rec", tag="rec")
            nc.vector.reciprocal_approx_fast(rec[:], pz[0:1, :])
            bc = nrm_pool.tile([65, 512], F32, name="bc", tag="bc")
            if mq != 3:
                # phase I: all these precede any collective on the gpsimd
                # queue, so the broadcast can't be stalled by one
                nc.gpsimd.partition_broadcast(bc[:], rec[0:1, :])
            else:
                # phase II: a pending collective_compute occupies gpsimd
                # until the collective completes (with wildly variable
                # duration), so broadcast via a DRAM round-trip on the sync
                # queue instead: rec -> DRAM, then a stride-0 partition AP
                # fans it out to 65 partitions
                recd = dram.tile([1, 512], F32, name="recd", tag="recd", bufs=4)
                nc.sync.dma_start(recd[:], rec[:])
                nc.sync.dma_start(
                    bc[0:65, :], recd[0:1, :].partition_broadcast(65)
                )
            zt = z_pool.tile([65, 512], BF16, name="ztile")
            # PSUM reads must start at partition 0: multiply the l row too
            # (l * rec = 1, discarded) and DMA only rows 1..64.
            nc.vector.tensor_mul(zt[0:65, :], pz[0:65, :], bc[0:65, :])
            return nc.sync.dma_start(buf[rank, :, :], zt[1:65, :])

        def emit_a2a(key):
            a_in, a_out = a2a[key]
            nc.gpsimd.collective_compute(
                "AllToAll",
                mybir.AluOpType.bypass,
                replica_groups=[list(range(NCORES))],
                ins=[a_in[:].opt()],
                outs=[a_out[:].opt()],
            )

        def emit_zrecv(key, tagp):
            """4 rank-pair packed tiles from ONE collective's output:
            pack[k] = [out[2k] ; out[2k+1]] -> global heads (4k+hh, 4k+2+hh).
            On the gpsimd DMA queue (not sync): keeps the z-DMA rings free of
            collective-gated entries, so the O-projection's LDW watermarks on
            these tiles release as soon as the packs land rather than after
            unrelated later ring traffic."""
            packs = []
            for k in range(4):
                t = zr_pool.tile(
                    [128, 512], BF16, name=f"zr{tagp}{k}", tag=f"zr{tagp}{k}"
                )
                nc.gpsimd.dma_start(t[0:64, :], a2a[key][1][2 * k, :, :])
                nc.gpsimd.dma_start(t[64:128, :], a2a[key][1][2 * k + 1, :, :])
                packs.append(t)
            return packs

        def emit_oproj(packs, row0):
            """packs = 8 tiles, contraction order matching host-packed otP."""
            for qc in range(4):
                ob = ob_pool.tile([128, 1024], F32, name="ob")
                pss = [
                    ps_gen.tile([128, 512], F32, name="pso", tag="gen")
                    for _ in range(2)
                ]
                for c in range(8):
                    for mh in range(2):
                        nc.tensor.matmul(
                            pss[mh][:],
                            packs[c][:, 128 * qc:128 * (qc + 1)],
                            ot_sb[c][:, 512 * mh:512 * (mh + 1)],
                            start=(c == 0),
                            stop=(c == 7),
                        )
                nc.vector.tensor_copy(ob[:, 0:512], pss[0][:])
                nc.scalar.activation(ob[:, 512:1024], pss[1][:], Copy)
                r = row0 + 128 * qc
                nc.scalar.dma_start(out_ext[r:r + 128, :], ob[:])

        # ---- phase I: all projections + attn for macros {0,1,2}. Keeping
        # attn {1} here (instead of a separate attention-only phase) hides
        # its ACT exp work under the PE-heavy projections. x DMAs for the
        # NEXT batch are emitted before this batch's attention so they
        # never queue behind softmax-dependent work on the in-order DMA
        # queues.
        pend = {}
        pend[(0, 0)] = emit_xt(0, 0, nc.scalar)
        pend[(0, 1)] = emit_xt(0, 1, nc.gpsimd)
        for b in range(B):
            emit_proj(b, 0, pend.pop((b, 0)))
            emit_proj(b, 1, pend.pop((b, 1)))
            if b + 1 < B:
                pend[(b + 1, 0)] = emit_xt(b + 1, 0)
                pend[(b + 1, 1)] = emit_xt(b + 1, 1)
            emit_attn(b, 0, 0)
            emit_attn(b, 0, 1)
            emit_attn(b, 1, 0)
            emit_attn(b, 1, 1)
        pend[(0, 2)] = emit_xt(0, 2)
        pend[(0, 3)] = emit_xt(0, 3)
        for b in range(B):
            emit_proj(b, 2, pend.pop((b, 2)))
            emit_proj(b, 3, pend.pop((b, 3)))
            if b + 1 < B:
                pend[(b + 1, 2)] = emit_xt(b + 1, 2)
                pend[(b + 1, 3)] = emit_xt(b + 1, 3)
            if b == B - 1:
                for m in range(8):
                    nc.sync.dma_start(
                        ot_sb[m][:], ot_ext[128 * m:128 * (m + 1), :]
                    )
            emit_attn(b, 2, 0)
            emit_attn(b, 2, 1)
        emit_a2a("1a")
        # ---- phase II: macro-3 attention, head A then head B ----
        for b in range(B):
            emit_attn(b, 3, 0, pz_pool=(ps_gen if b < 2 else ps_z))
            if b == 0:
                emit_a2a("1b")
            if b == 1:
                # gpsimd reaches these after CC-1b's completion, by which
                # point 1a/1b data is ready: no long head-block, and the
                # packs land well before the O-projection needs them
                packs_1a = emit_zrecv("1a", "p1a")
                packs_1b = emit_zrecv("1b", "p1b")
        emit_a2a("2a")
        for b in range(B):
            emit_attn(b, 3, 1, pz_pool=(ps_gen if b < 2 else ps_z))
            if b == 0:
                packs_2a = emit_zrecv("2a", "p2a")
        emit_a2a("2b")
        packs_2b = emit_zrecv("2b", "p2b")
        # ---- O-projection. Rows 0..511 overlap the 2b exchange; rows
        # 512..1023 start on the 2a-half packs while 2b drains.
        emit_oproj(packs_1a + packs_1b, 0)
        emit_oproj(packs_2a + packs_2b, CH2)

    nc.compile()
    return nc


_BUILT = {}


def _get_built():
    if "nc" not in _BUILT:
        _BUILT["nc"] = build()
    return _BUILT["nc"]


def prep_inputs(x, Q, K, V, O):
    x = np.asarray(x, dtype=np.float32)
    Q = np.asarray(Q, dtype=np.float32)
    K = np.asarray(K, dtype=np.float32)
    V = np.asarray(V, dtype=np.float32)
    O = np.asarray(O, dtype=np.float32)
    xt = np.ascontiguousarray(np.transpose(x, (0, 2, 1))).astype(np.float16)  # [B, M, S]
    # O.T rows a = n*64 + h, regrouped to match the rank-pair packed zrecv
    # tiles: chunk k<4 = heads (4k, 4k+2) [head-A of ranks 2k,2k+1], chunk
    # k>=4 = heads (4k'+1, 4k'+3) [head-B].
    otf = O.T.reshape(16, 64, 1024)
    perm = [0, 2, 4, 6, 8, 10, 12, 14, 1, 3, 5, 7, 9, 11, 13, 15]
    ot = np.ascontiguousarray(otf[perm].reshape(1024, 1024)).astype(ml_dtypes.bfloat16)
    p = np.arange(128)
    msk = np.concatenate(
        [
            (p[:, None] <= p[None, :]).astype(np.float16),          # tri
            np.where(p[:, None] > p[None, :], np.float16(-1000), np.float16(0)),
        ],
        axis=1,
    )  # [128, 256]
    in_maps = []
    for j in range(NCORES):
        hA, hB = 2 * j, 2 * j + 1
        wqk = np.ascontiguousarray(
            np.concatenate([Q[hA], Q[hB], K[hA], K[hB]], axis=0).T
        ).astype(np.float16)  # [1024, 256]
        wv = np.ascontiguousarray(
            np.concatenate([V[hA], V[hB]], axis=0).T
        ).astype(np.float16)  # [1024, 128]
        in_maps.append({"xt": xt, "wqk": wqk, "wv": wv, "ot": ot, "msk": msk})
    return in_maps


LAST_EXEC_TIME_NS = None


def kernel(x, Q, K, V, O):
    global LAST_EXEC_TIME_NS
    x = np.asarray(x)
    assert x.shape[1] == S
    nc = _get_built()
    in_maps = prep_inputs(x, Q, K, V, O)
    trace = bool(int(os.environ.get("ATTN_TRACE", "0")))
    res = run_bass_kernel_spmd(nc, in_maps, list(range(NCORES)), trace=trace)
    LAST_EXEC_TIME_NS = res.exec_time_ns
    out = np.zeros((B, S, M), np.float32)
    half = S // 2
    for j in range(NCORES):
        b, hh = j // 2, j % 2
        out[b, hh * half:(hh + 1) * half, :] = res.results[j]["out"]
    return out


# revision 30
# speedup vs baseline: 1.0334x; 1.0334x over previous
"""Distributed causal multi-head attention for 8 TRN2 NeuronCores.

Problem: x[4,2048,1024], per-head Q/K/V [16,64,1024], O [1024,1024].
  q,k,v = per-head projections of x; scores = q@k^T (no 1/sqrt(d));
  causal softmax; z = attn@v; out = z @ O^T.

Sharding (head-parallel): core j owns heads {2j, 2j+1} for ALL batches.

Structure (vs the 2-A2A baseline):
  - Single-head attention units (b, mq, h) so the z-exchange splits into
    FOUR AllToAlls (512 KB each: 1a/1b = macros {0,2} heads A/B, 2a/2b =
    macros {1,3}): 1a/1b hide under phase II head-A attention, 2a hides
    under phase II head-B, 2b hides under the O-projection of rows 0-511.
  - Causal diagonal trimmed at 128-col granularity: scores/PV matmuls and
    exp only cover q >= 128*d within the diagonal 512-block.
  - Causal mask applied IN PSUM via one extra 128-col matmul per diagonal
    k-tile: tri[p,i]=[p<=i] (stationary) x mneg[p,j]=-1000*[p>j] (moving)
    accumulates -1000*max(0,i-j) onto the boundary subtile, so exp
    underflows to exactly 0 above the diagonal. No gpsimd affine_select
    on the critical path.
  - PV ones-column trick with l at psum partition 0 (v tile layout
    [1|vA|1|vB]): reciprocal runs directly on pz[0:1,:] (DVE reads psum,
    offsets match), no l-row DMA, no zcp copy.
  - PV of group g-1 is emitted after scores+exp of group g (software
    pipelining) so the PE never waits on the exp of the group it just
    computed.
"""

import os

import numpy as np
import ml_dtypes

import concourse.mybir as mybir
import concourse.tile as tile
from concourse import bacc
from concourse.bass_utils import run_bass_kernel_spmd

BF16 = mybir.dt.bfloat16
F32 = mybir.dt.float32
FP16 = mybir.dt.float16

B, M, NH, DH = 4, 1024, 16, 64
NCORES = 8
S = 2048
NM = S // 512          # 512-wide q-macros per batch
CH = (B * S) // NCORES  # output rows per core (1024)
CH2 = CH // 2

Exp = mybir.ActivationFunctionType.Exp
Copy = mybir.ActivationFunctionType.Copy


def build():
    nc = bacc.Bacc("TRN2", target_bir_lowering=False, debug=False, num_devices=NCORES)
    xt_ext = nc.dram_tensor("xt", [B, M, S], FP16, kind="ExternalInput")
    wqk_ext = nc.dram_tensor("wqk", [M, 256], FP16, kind="ExternalInput")
    wv_ext = nc.dram_tensor("wv", [M, 128], FP16, kind="ExternalInput")
    ot_ext = nc.dram_tensor("ot", [M, M], BF16, kind="ExternalInput")
    msk_ext = nc.dram_tensor("msk", [128, 256], FP16, kind="ExternalInput")
    out_ext = nc.dram_tensor("out", [CH, M], F32, kind="ExternalOutput")

    with (
        tile.TileContext(nc) as tc,
        tc.tile_pool(name="wpool", bufs=1) as wpool,
        tc.tile_pool(name="xt", bufs=18) as xt_pool,
        tc.tile_pool(name="qk", bufs=1) as qk_pool,
        tc.tile_pool(name="kz", bufs=1) as kz_pool,
        tc.tile_pool(name="vp", bufs=1) as v_pool,
        tc.tile_pool(name="ep", bufs=6) as e_pool,
        tc.tile_pool(name="zp", bufs=6) as z_pool,
        tc.tile_pool(name="zr", bufs=1) as zr_pool,
        tc.tile_pool(name="ob", bufs=2) as ob_pool,
        tc.tile_pool(name="nrm", bufs=3) as nrm_pool,
        tc.tile_pool(name="ps_sc", bufs=2, space="PSUM") as ps_sc,
        tc.tile_pool(name="ps_z", bufs=2, space="PSUM") as ps_z,
        tc.tile_pool(name="ps_gen", bufs=2, space="PSUM") as ps_gen,
        tc.tile_pool(name="dram", bufs=1, space="DRAM") as dram,
    ):
        # ---- weights (resident) ----
        wqk_sb, wv_sb, ot_sb = [], [], []
        for m in range(8):
            t = wpool.tile([128, 256], FP16, name=f"wqk{m}", tag=f"wqk{m}")
            nc.sync.dma_start(t[:], wqk_ext[128 * m:128 * (m + 1), :])
            wqk_sb.append(t)
            t = wpool.tile([128, 128], FP16, name=f"wv{m}", tag=f"wv{m}")
            nc.gpsimd.dma_start(t[:], wv_ext[128 * m:128 * (m + 1), :])
            wv_sb.append(t)
            t = wpool.tile([128, 1024], BF16, name=f"ot{m}", tag=f"ot{m}")
            ot_sb.append(t)

        # ---- causal-mask constant tiles (host-prepared) ----
        # tri[p,i] = 1 if p <= i ; mneg[p,j] = -1000 if p > j else 0.
        # matmul(tri, mneg) accumulates -1000*max(0, i-j) onto scores psum.
        msk = wpool.tile([128, 256], FP16, name="msk", tag="msk")
        nc.gpsimd.dma_start(msk[:], msk_ext[:, :])
        tri = msk[:, 0:128]
        mneg = msk[:, 128:256]

        # ---- exchange buffers: [rank, 64 v-dims, 512 q] per head/phase ----
        def dbuf(name):
            return dram.tile([NCORES, 64, 512], BF16, name=name)

        a2a = {k: (dbuf(f"in_{k}"), dbuf(f"out_{k}"))
               for k in ("1a", "1b", "2a", "2b")}

        qk_sb = {}   # (b, mq) -> [128, 512] fp16 qT, rows = (qA dims | qB dims)
        kz_sb = {}   # (h, b, mk) -> [128, 512] fp16 zero-padded per-head kT
        v_sb = {}    # (b, kt) -> [128, 130] bf16: [1 | vA(64) | 1 | vB(64)]

        def emit_xt(b, mq, xt_eng=None):
            xts = []
            for m in range(8):
                t = xt_pool.tile([128, 512], FP16, name="xtc")
                eng = xt_eng if xt_eng is not None else nc.sync
                eng.dma_start(
                    t[:], xt_ext[b, 128 * m:128 * (m + 1), 512 * mq:512 * (mq + 1)]
                )
                xts.append(t)
            return xts

        def _emit_v_tile(b, mq, stl, ps):
            kt = 4 * mq + stl
            # [1 | vA(64) | 1 | vB(64)]: ones column -> PV psum row 0 = l,
            # z dims land on psum rows 1..64
            vt = v_pool.tile([128, 130], BF16, name=f"v_{b}_{kt}", tag=f"v_{b}_{kt}")
            nc.gpsimd.memset(vt[:, 0:1], 1.0)
            nc.gpsimd.memset(vt[:, 65:66], 1.0)
            nc.vector.tensor_copy(
                vt[:].rearrange("p (g c) -> p g c", g=2)[:, :, 1:65],
                ps[:].rearrange("p (g c) -> p g c", g=2),
            )
            v_sb[(b, kt)] = vt

        def emit_proj(b, mq, xts):
            # interleave the 512-wide qk matmuls with the 128-wide v matmuls
            # so each one's LDWEIGHTS hides under the other's execution
            for ct in range(2):
                ps = ps_gen.tile([128, 512], F32, name="psqk", tag="gen")
                psv = ps_gen.tile([128, 128], F32, name="psv", tag="gen")
                for m in range(8):
                    nc.tensor.matmul(
                        ps[:],
                        wqk_sb[m][:, 128 * ct:128 * (ct + 1)],
                        xts[m][:],
                        start=(m == 0),
                        stop=(m == 7),
                    )
                    nc.tensor.matmul(
                        psv[:],
                        xts[m][:, 128 * ct:128 * (ct + 1)],
                        wv_sb[m][:],
                        start=(m == 0),
                        stop=(m == 7),
                    )
                _emit_v_tile(b, mq, ct, psv)
                if ct == 0:
                    t = qk_pool.tile(
                        [128, 512], FP16, name=f"qk_{b}_{mq}", tag=f"qk_{b}_{mq}"
                    )
                    nc.vector.tensor_copy(t[:], ps[:])
                    qk_sb[(b, mq)] = t
                else:
                    for h in range(2):
                        kz = kz_pool.tile(
                            [128, 512], FP16, name=f"kz{h}_{b}_{mq}",
                            tag=f"kz{h}_{b}_{mq}",
                        )
                        nc.vector.memset(kz[64 - 64 * h:128 - 64 * h, :], 0.0)
                        nc.scalar.activation(
                            kz[64 * h:64 * (h + 1), :],
                            ps[64 * h:64 * (h + 1), :],
                            Copy,
                        )
                        kz_sb[(h, b, mq)] = kz
            for stl in (2, 3):
                ps = ps_gen.tile([128, 128], F32, name="psv", tag="gen")
                for m in range(8):
                    nc.tensor.matmul(
                        ps[:],
                        xts[m][:, 128 * stl:128 * (stl + 1)],
                        wv_sb[m][:],
                        start=(m == 0),
                        stop=(m == 7),
                    )
                _emit_v_tile(b, mq, stl, ps)

        def emit_attn(b, mq, h, pz_pool=None):
            """Single-head attention unit -> zt DMA into its exchange slice.

            pz_pool: the macro-3 phase (no projections running) borrows the
            idle ps_gen banks for half its pz tiles, giving a 4-deep pz
            rotation. A collective_compute occupies the gpsimd queue until
            the collective completes, stalling partition_broadcasts of the
            following units; the deeper rotation keeps the PE fed meanwhile.
            """
            nk = 4 * (mq + 1)
            buf = a2a[("1" if mq % 2 == 0 else "2") + ("a" if h == 0 else "b")][0]
            rank = 2 * b + mq // 2
            if pz_pool is None:
                pz_pool = ps_z
            pz = pz_pool.tile(
                [128, 512], F32, name="pz",
                tag="pz" if pz_pool is ps_z else "gen",
            )
            qk = qk_sb[(b, mq)]
            pending = None

            def emit_pv(p):
                e, spans = p
                for kt, kk, q0 in spans:
                    vt = v_sb[(b, kt)]
                    nc.tensor.matmul(
                        pz[0:65, q0:512],
                        vt[:, 65 * h:65 * h + 65],
                        e[:, 512 * kk + q0:512 * (kk + 1)],
                        start=(kt == 0),
                        stop=(kt == nk - 1),
                        skip_group_check=True,
                    )

            for g in range(nk // 2):
                psc = ps_sc.tile([128, 1024], F32, name="psc", tag="sc")
                spans = []
                for kk in range(2):
                    kt = 2 * g + kk
                    mk, ktl = kt // 4, kt % 4
                    d = kt - 4 * mq
                    q0 = 128 * d if d >= 0 else 0
                    nc.tensor.matmul(
                        psc[:, 512 * kk + q0:512 * (kk + 1)],
                        kz_sb[(h, b, mk)][:, 128 * ktl:128 * (ktl + 1)],
                        qk[:, q0:512],
                        start=True,
                        stop=(d < 0),
                        skip_group_check=True,
                    )
                    if d >= 0:
                        # subtract 1000*(k_local - q_local) on the boundary
                        # 128x128 subtile; exp underflows to exact 0 there
                        nc.tensor.matmul(
                            psc[:, 512 * kk + q0:512 * kk + q0 + 128],
                            tri,
                            mneg,
                            start=False,
                            stop=True,
                            skip_group_check=True,
                        )
                    spans.append((kt, kk, q0))
                e = e_pool.tile([128, 1024], BF16, name="etile")
                # one exp per group when the mid-tile garbage span is small
                # (cheaper than a second instruction's fixed cost)
                if spans[1][2] <= 128:
                    q0a = spans[0][2]
                    nc.scalar.activation(e[:, q0a:1024], psc[:, q0a:1024], Exp)
                else:
                    for kt, kk, q0 in spans:
                        nc.scalar.activation(
                            e[:, 512 * kk + q0:512 * (kk + 1)],
                            psc[:, 512 * kk + q0:512 * (kk + 1)],
                            Exp,
                        )
                if pending is not None:
                    emit_pv(pending)
                pending = (e, spans)
            emit_pv(pending)

            # normalize: l = pz row 0 (psum partition 0); rec/bc/mul with
            # matching partition offsets throughout
            rec = nrm_pool.tile([1, 512], F32, name="rec", tag="rec")
            nc.vector.reciprocal_approx_fast(rec[:], pz[0:1, :])
            bc = nrm_pool.tile([65, 512], F32, name="bc", tag="bc")
            if mq != 3:
                # phase I: all these precede any collective on the gpsimd
                # queue, so the broadcast can't be stalled by one
                nc.gpsimd.partition_broadcast(bc[:], rec[0:1, :])
            else:
                # phase II: a pending collective_compute occupies gpsimd
                # until the collective completes (with wildly variable
                # duration), so broadcast via a DRAM round-trip on the sync
                # queue instead: rec -> DRAM, then a stride-0 partition AP
                # fans it out to 65 partitions
                recd = dram.tile([1, 512], F32, name="recd", tag="recd", bufs=4)
                nc.sync.dma_start(recd[:], rec[:])
                nc.sync.dma_start(
                    bc[0:65, :], recd[0:1, :].partition_broadcast(65)
                )
            zt = z_pool.tile([65, 512], BF16, name="ztile")
            # PSUM reads must start at partition 0: multiply the l row too
            # (l * rec = 1, discarded) and DMA only rows 1..64.
            nc.vector.tensor_mul(zt[0:65, :], pz[0:65, :], bc[0:65, :])
            return nc.sync.dma_start(buf[rank, :, :], zt[1:65, :])

        def emit_a2a(key):
            a_in, a_out = a2a[key]
            nc.gpsimd.collective_compute(
                "AllToAll",
                mybir.AluOpType.bypass,
                replica_groups=[list(range(NCORES))],
                ins=[a_in[:].opt()],
                outs=[a_out[:].opt()],
            )

        def emit_zrecv(key, tagp):
            """4 rank-pair packed tiles from ONE collective's output:
            pack[k] = [out[2k] ; out[2k+1]] -> global heads (4k+hh, 4k+2+hh).
            On the gpsimd DMA queue (not sync): keeps the z-DMA rings free of
            collective-gated entries, so the O-projection's LDW watermarks on
            these tiles release as soon as the packs land rather than after
            unrelated later ring traffic."""
            packs = []
            for k in range(4):
                t = zr_pool.tile(
                    [128, 512], BF16, name=f"zr{tagp}{k}", tag=f"zr{tagp}{k}"
                )
                nc.gpsimd.dma_start(t[0:64, :], a2a[key][1][2 * k, :, :])
                nc.gpsimd.dma_start(t[64:128, :], a2a[key][1][2 * k + 1, :, :])
                packs.append(t)
            return packs

        def emit_oproj(packs, row0):
            """packs = 8 tiles, contraction order matching host-packed otP."""
            for qc in range(4):
                ob = ob_pool.tile([128, 1024], F32, name="ob")
                pss = [
                    ps_gen.tile([128, 512], F32, name="pso", tag="gen")
                    for _ in range(2)
                ]
                for c in range(8):
                    for mh in range(2):
                        nc.tensor.matmul(
                            pss[mh][:],
                            packs[c][:, 128 * qc:128 * (qc + 1)],
                            ot_sb[c][:, 512 * mh:512 * (mh + 1)],
                            start=(c == 0),
                            stop=(c == 7),
                        )
                nc.vector.tensor_copy(ob[:, 0:512], pss[0][:])
                nc.scalar.activation(ob[:, 512:1024], pss[1][:], Copy)
                r = row0 + 128 * qc
                nc.scalar.dma_start(out_ext[r:r + 128, :], ob[:])

        # ---- phase I: all projections + attn for macros {0,1,2}. Keeping
        # attn {1} here (instead of a separate attention-only phase) hides
        # its ACT exp work under the PE-heavy projections. x DMAs for the
        # NEXT batch are emitted before this batch's attention so they
        # never queue behind softmax-dependent work on the in-order DMA
        # queues.
        pend = {}
        pend[(0, 0)] = emit_xt(0, 0, nc.scalar)
        pend[(0, 1)] = emit_xt(0, 1, nc.gpsimd)
        for b in range(B):
            emit_proj(b, 0, pend.pop((b, 0)))
            emit_proj(b, 1, pend.pop((b, 1)))
            if b + 1 < B:
                pend[(b + 1, 0)] = emit_xt(b + 1, 0)
                pend[(b + 1, 1)] = emit_xt(b + 1, 1)
            emit_attn(b, 0, 0)
            emit_attn(b, 0, 1)
            emit_attn(b, 1, 0)
            emit_attn(b, 1, 1)
        pend[(0, 2)] = emit_xt(0, 2)
        pend[(0, 3)] = emit_xt(0, 3)
        for b in range(B):
            emit_proj(b, 2, pend.pop((b, 2)))
            emit_proj(b, 3, pend.pop((b, 3)))
            if b + 1 < B:
                pend[(b + 1, 2)] = emit_xt(b + 1, 2)
                pend[(b + 1, 3)] = emit_xt(b + 1, 3)
            if b == B - 1:
                for m in range(8):
                    nc.sync.dma_start(
                        ot_sb[m][:], ot_ext[128 * m:128 * (m + 1), :]
                    )
            emit_attn(b, 2, 0)
            emit_attn(b, 2, 1)
        emit_a2a("1a")
        # ---- phase II: macro-3 attention, head A then head B ----
        for b in range(B):
            emit_attn(b, 3, 0, pz_pool=(ps_gen if b < 2 else ps_z))
            if b == 0:
                emit_a2a("1b")
            if b == 1:
                # gpsimd reaches these after CC-1b's completion, by which
                # point 1a/1b data is ready: no long head-block, and the
                # packs land well before the O-projection needs them
                packs_1a = emit_zrecv("1a", "p1a")
                packs_1b = emit_zrecv("1b", "p1b")
        emit_a2a("2a")
        for b in range(B):
            emit_attn(b, 3, 1, pz_pool=(ps_gen if b < 2 else ps_z))
            if b == 0:
                packs_2a = emit_zrecv("2a", "p2a")
        # The LDW->DMA watermark the framework emits waits for the producer
        # DMA plus its successor on the same gpsimd DMA ring. Without these
        # dummies, the ring-successors of the packs_1b/packs_2a entries are
        # 2b-gated packs_2b DMAs, which would stall the whole O-projection
        # until the LAST collective completes. One tiny DMA per ring makes
        # every pack's successor complete immediately.
        dummy = nrm_pool.tile([1, 16], FP16, name="dummy", tag="dummy")
        for k in range(8):
            nc.gpsimd.dma_start(dummy[0:1, 2 * k:2 * k + 2], msk_ext[0:1, 0:2])
        emit_a2a("2b")
        packs_2b = emit_zrecv("2b", "p2b")
        # ---- O-projection. Rows 0..511 overlap the 2b exchange; rows
        # 512..1023 start on the 2a-half packs while 2b drains.
        emit_oproj(packs_1a + packs_1b, 0)
        emit_oproj(packs_2a + packs_2b, CH2)

    nc.compile()
    return nc


_BUILT = {}


def _get_built():
    if "nc" not in _BUILT:
        _BUILT["nc"] = build()
    return _BUILT["nc"]


def prep_inputs(x, Q, K, V, O):
    x = np.asarray(x, dtype=np.float32)
    Q = np.asarray(Q, dtype=np.float32)
    K = np.asarray(K, dtype=np.float32)
    V = np.asarray(V, dtype=np.float32)
    O = np.asarray(O, dtype=np.float32)
    xt = np.ascontiguousarray(np.transpose(x, (0, 2, 1))).astype(np.float16)  # [B, M, S]
    # O.T rows a = n*64 + h, regrouped to match the rank-pair packed zrecv
    # tiles: chunk k<4 = heads (4k, 4k+2) [head-A of ranks 2k,2k+1], chunk
    # k>=4 = heads (4k'+1, 4k'+3) [head-B].
    otf = O.T.reshape(16, 64, 1024)
    perm = [0, 2, 4, 6, 8, 10, 12, 14, 1, 3, 5, 7, 9, 11, 13, 15]
    ot = np.ascontiguousarray(otf[perm].reshape(1024, 1024)).astype(ml_dtypes.bfloat16)
    p = np.arange(128)
    msk = np.concatenate(
        [
            (p[:, None] <= p[None, :]).astype(np.float16),          # tri
            np.where(p[:, None] > p[None, :], np.float16(-1000), np.float16(0)),
        ],
        axis=1,
    )  # [128, 256]
    in_maps = []
    for j in range(NCORES):
        hA, hB = 2 * j, 2 * j + 1
        wqk = np.ascontiguousarray(
            np.concatenate([Q[hA], Q[hB], K[hA], K[hB]], axis=0).T
        ).astype(np.float16)  # [1024, 256]
        wv = np.ascontiguousarray(
            np.concatenate([V[hA], V[hB]], axis=0).T
        ).astype(np.float16)  # [1024, 128]
        in_maps.append({"xt": xt, "wqk": wqk, "wv": wv, "ot": ot, "msk": msk})
    return in_maps


LAST_EXEC_TIME_NS = None


def kernel(x, Q, K, V, O):
    global LAST_EXEC_TIME_NS
    x = np.asarray(x)
    assert x.shape[1] == S
    nc = _get_built()
    in_maps = prep_inputs(x, Q, K, V, O)
    trace = bool(int(os.environ.get("ATTN_TRACE", "0")))
    res = run_bass_kernel_spmd(nc, in_maps, list(range(NCORES)), trace=trace)
    LAST_EXEC_TIME_NS = res.exec_time_ns
    out = np.zeros((B, S, M), np.float32)
    half = S // 2
    for j in range(NCORES):
        b, hh = j // 2, j % 2
        out[b, hh * half:(hh + 1) * half, :] = res.results[j]["out"]
    return out
